# revision 1
# baseline (speedup 1.0000x reference)
"""Energy-score loss kernel for Trainium2 (8 NeuronCores, SPMD over latlon axis).

Full inputs in, full (scalar) output out. The latlon axis (L=40320) is
sharded 8 ways (5040 per core, zero-padded to 5120). Each core computes
partial sums of nw_l * (7*sum_i prec_i(l) - sum_{i<j} dist_ij(l)) over its
shard; the host divides by (56 * sum(nw) * B).

Rotation-gram algorithm, per (b, 512-latlon super chunk), partitions hold
128 latlon points, free axis holds (chunk c in 0..3, feature v in 0..127):

  Y[i] = (preds_i | target) * feature_weights         9 wide mults, bf16 out
  For rotation k = 0..4:   (pairs (i, (i+k) mod 9) cover all 45 dots)
    P_k = Y * rot_k(Y)                                 bf16 2x wide mults
    Q[., 9k+i] = sum_v P_k  via fold tree + reduce     fp32 accumulation
  d2[k', i] = Q_ii + Q_(i+k') - 2*Q_pair               3 wide TT ops
  dist = sqrt(max(d2, 0))                              1 ACT op
  prec = dist columns involving the target (affine views)
  acc += nw * (8*sum(prec) - sum(dist))
"""

import numpy as np

import concourse.bass as bass
import concourse.tile as tile
from concourse import mybir
from concourse.bacc import Bacc
from concourse.bass_utils import run_bass_kernel_spmd

# Problem constants (hardcoded per contract).
B, M, L, V = 2, 8, 40320, 128
NCORES = 8
LSH = L // NCORES          # 5040 latlon per core
LPAD = 5120                # padded to a multiple of 512
P = 128                    # partitions per chunk
SC = 512                   # latlon super-chunk
NSC = LPAD // SC           # 10 super chunks per b
CPS = SC // P              # 4 chunks per super chunk
NVEC = 9                   # 8 ensemble members + target
NROT = 5                   # rotations 0..4 cover all 45 unordered pairs

F32 = mybir.dt.float32
BF16 = mybir.dt.bfloat16


def _ap(base, dims):
    """Manual AP view of a tile: dims = [(stride, size), ...] in elements,
    first entry is the partition dim (copied from base)."""
    return bass.AP(tensor=base.tensor, offset=base.offset, ap=[list(d) for d in dims])


def build_bass(loop_n=None):
    nc = Bacc(None)
    preds = nc.declare_dram_parameter("preds", (B, M, LPAD, V), F32, isOutput=False)
    target = nc.declare_dram_parameter("target", (B, LPAD, V), F32, isOutput=False)
    fw = nc.declare_dram_parameter("fw", (V,), F32, isOutput=False)
    nw = nc.declare_dram_parameter("nw", (LPAD,), F32, isOutput=False)
    out = nc.declare_dram_parameter("out", (B, P, NSC, CPS), F32, isOutput=True)

    with tile.TileContext(nc) as tc:
        with (
            tc.tile_pool(name="singles", bufs=1) as singles,
            tc.tile_pool(name="xin", bufs=3) as xin,
            tc.tile_pool(name="ysc", bufs=2) as ysc,
            tc.tile_pool(name="rot", bufs=2) as rot,
            tc.tile_pool(name="folds", bufs=2) as folds,
            tc.tile_pool(name="qpool", bufs=2) as qpool,
            tc.tile_pool(name="smalls", bufs=4) as smalls,
        ):
            # feature weights broadcast to all partitions: tile[p, v] = fw[v]
            fw_sb = singles.tile([P, V], F32)
            nc.sync.dma_start(out=fw_sb, in_=fw[:].partition_broadcast(P))

            # node weights, interleaved layout: tile[p, s, c] = nw[s*512 + p*4 + c]
            nw_sb = singles.tile([P, NSC, CPS], F32)
            nc.sync.dma_start(
                out=nw_sb, in_=nw[:].rearrange("(s p c) -> p s c", p=P, c=CPS)
            )

            # per-(b, sc, c) weighted-energy partials; summed on host
            e1_all = singles.tile([P, B, NSC, CPS], F32)

            # absorb preamble DMA queue ticks into the DVE clock (1-wait limit)
            junk = singles.tile([P, 2], F32)
            nc.vector.tensor_copy(junk[:, 0:1], fw_sb[:, 0:1])
            nc.vector.tensor_copy(junk[:, 1:2], nw_sb[:, 0, 0:1])

            import contextlib
            loop_ctx = tc.For_i(0, loop_n, 1) if loop_n else contextlib.nullcontext()
            with loop_ctx:
              for b in range(B):
                for sc in range(NSC):
                    l0 = sc * SC
                    # Load 9 vectors for 512 latlon rows, one 2D DMA each
                    # (2KB contiguous per partition): latlon l = l0 + p*4 + c.
                    xs = [
                        xin.tile([P, CPS, V], F32, tag=f"x{i}", name=f"x{i}_{b}_{sc}")
                        for i in range(NVEC)
                    ]
                    for i in range(M):
                        nc.sync.dma_start(
                            out=xs[i],
                            in_=preds[b, i, l0 : l0 + SC, :].rearrange(
                                "(p c) v -> p c v", p=P
                            ),
                        )
                    nc.sync.dma_start(
                        out=xs[8],
                        in_=target[b, l0 : l0 + SC, :].rearrange(
                            "(p c) v -> p c v", p=P
                        ),
                    )

                    # scale by feature weights (broadcast along c), cast to bf16
                    y = ysc.tile([P, NVEC, CPS, V], BF16)
                    fw_b = _ap(fw_sb, [fw_sb.ap[0], [0, CPS], fw_sb.ap[1]])
                    for i in range(NVEC):
                        nc.vector.tensor_tensor(
                            out=y[:, i], in0=xs[i], in1=fw_b,
                            op=mybir.AluOpType.mult,
                        )

                    # Q[p, c, 9k+i] = <y_i, y_(i+k mod 9)> for chunk c
                    q = qpool.tile([P, CPS, NROT * NVEC], F32)
                    for k in range(NROT):
                        pk = rot.tile([P, NVEC, CPS, V], BF16, tag="pk",
                                      name=f"pk_{b}_{sc}_{k}")
                        if k == 0:
                            # self-squares on the (otherwise idle) scalar engine
                            nc.scalar.activation(
                                out=pk, in_=y,
                                func=mybir.ActivationFunctionType.Square,
                            )
                        else:
                            nc.vector.tensor_tensor(
                                out=pk[:, 0 : NVEC - k],
                                in0=y[:, 0 : NVEC - k],
                                in1=y[:, k:NVEC],
                                op=mybir.AluOpType.mult,
                            )
                            nc.vector.tensor_tensor(
                                out=pk[:, NVEC - k : NVEC],
                                in0=y[:, NVEC - k : NVEC],
                                in1=y[:, 0:k],
                                op=mybir.AluOpType.mult,
                            )
                        # fold tree over v: 128 -> 64 -> 32 -> 16, then reduce
                        f1 = folds.tile([P, NVEC, CPS, 64], BF16, tag="f1")
                        nc.vector.tensor_tensor(
                            out=f1, in0=pk[:, :, :, 0:64], in1=pk[:, :, :, 64:128],
                            op=mybir.AluOpType.add,
                        )
                        f2 = folds.tile([P, NVEC, CPS, 32], BF16, tag="f2")
                        nc.vector.tensor_tensor(
                            out=f2, in0=f1[:, :, :, 0:32], in1=f1[:, :, :, 32:64],
                            op=mybir.AluOpType.add,
                        )
                        f3 = folds.tile([P, NVEC, CPS, 16], BF16, tag="f3")
                        nc.vector.tensor_tensor(
                            out=f3, in0=f2[:, :, :, 0:16], in1=f2[:, :, :, 16:32],
                            op=mybir.AluOpType.add,
                        )
                        f4 = folds.tile([P, NVEC, CPS, 8], BF16, tag="f4")
                        nc.vector.tensor_tensor(
                            out=f4, in0=f3[:, :, :, 0:8], in1=f3[:, :, :, 8:16],
                            op=mybir.AluOpType.add,
                        )
                        # reduce (P, 9, CPS, 8) -> (P, 9, CPS) into q[:, :, 9k+i]
                        q_view = q.rearrange(
                            "p c (kk i) -> p kk i c", kk=NROT
                        )[:, k]
                        nc.vector.tensor_reduce(
                            out=q_view,
                            in_=f4,
                            axis=mybir.AxisListType.X,
                            op=mybir.AluOpType.add,
                        )

                    # selfsD[p, c, 0:18] = selfs twice (for rotated windows)
                    selfsD = smalls.tile([P, CPS, 2, NVEC], F32, tag="selfsD")
                    sin = _ap(q, [q.ap[0], [NROT * NVEC, CPS], [0, 2], [1, NVEC]])
                    nc.scalar.copy(selfsD, sin)

                    # d2[p, c, k', i] = Q_ii + Q_(i+k'+1 mod 9) - 2*Q_pair(k'+1, i)
                    d2 = smalls.tile([P, CPS, NROT - 1, NVEC], F32, tag="d2")
                    a_view = _ap(
                        q, [q.ap[0], [NROT * NVEC, CPS], [0, NROT - 1], [1, NVEC]]
                    )
                    b_view = _ap(
                        selfsD,
                        [selfsD.ap[0], [2 * NVEC, CPS], [1, NROT - 1], [1, NVEC]],
                    )
                    b_view = bass.AP(
                        tensor=b_view.tensor, offset=b_view.offset + 1,
                        ap=[list(d) for d in b_view.ap],
                    )
                    nc.vector.tensor_tensor(
                        out=d2, in0=a_view, in1=b_view, op=mybir.AluOpType.add
                    )
                    c_view = q[:, :, NVEC : NROT * NVEC].rearrange(
                        "p c (k i) -> p c k i", k=NROT - 1
                    )
                    # d2 = (Q_pair * -2) + (Q_ii + Q_jj)
                    nc.vector.scalar_tensor_tensor(
                        out=d2, in0=c_view, scalar=-2.0, in1=d2,
                        op0=mybir.AluOpType.mult, op1=mybir.AluOpType.add,
                    )
                    # clamp tiny negatives (DVE: keeps the d2->sqrt chain to
                    # one cross-engine hop; ACT Relu here measured slower)
                    nc.vector.tensor_scalar_max(d2, d2, 0.0)

                    # dist = sqrt(d2)
                    dist = smalls.tile([P, CPS, NROT - 1, NVEC], F32, tag="dist")
                    nc.scalar.activation(
                        out=dist, in_=d2, func=mybir.ActivationFunctionType.Sqrt
                    )

                    # row sums: total, and the 8 target-pair columns
                    tot = smalls.tile([P, CPS], F32, tag="tot")
                    nc.vector.tensor_reduce(
                        out=tot,
                        in_=dist.rearrange("p c k i -> p c (k i)"),
                        axis=mybir.AxisListType.X,
                        op=mybir.AluOpType.add,
                    )
                    precA = smalls.tile([P, CPS], F32, tag="precA")
                    nc.vector.tensor_reduce(
                        out=precA,
                        in_=dist[:, :, :, 8],
                        axis=mybir.AxisListType.X,
                        op=mybir.AluOpType.add,
                    )
                    # anti-diagonal (k', i=7-k'): flat col 8k'+7 within (k,i) grid
                    precB = smalls.tile([P, CPS], F32, tag="precB")
                    pb_view = bass.AP(
                        tensor=dist.tensor, offset=dist.offset + 7,
                        ap=[
                            list(dist.ap[0]),
                            [(NROT - 1) * NVEC, CPS],
                            [NVEC - 1, NROT - 1],
                        ],
                    )
                    nc.vector.tensor_reduce(
                        out=precB,
                        in_=pb_view,
                        axis=mybir.AxisListType.X,
                        op=mybir.AluOpType.add,
                    )
                    prec = smalls.tile([P, CPS], F32, tag="prec")
                    nc.vector.tensor_tensor(
                        out=prec, in0=precA, in1=precB, op=mybir.AluOpType.add
                    )
                    # 56*e = 7*prec_sum - spread_sum = 8*prec - tot
                    nc.vector.scalar_tensor_tensor(
                        out=e1_all[:, b, sc], in0=prec, scalar=8.0, in1=tot,
                        op0=mybir.AluOpType.mult, op1=mybir.AluOpType.subtract,
                    )

            # one weighted multiply for all (b, sc, c) at the end
            ew = singles.tile([P, B, NSC, CPS], F32)
            nw_bview = _ap(
                nw_sb, [nw_sb.ap[0], [0, B], [CPS, NSC], [1, CPS]]
            )
            nc.vector.tensor_tensor(
                out=ew, in0=e1_all, in1=nw_bview, op=mybir.AluOpType.mult
            )
            for b in range(B):
                nc.sync.dma_start(out=out[b], in_=ew[:, b])

    nc.compile()
    return nc


_NC_CACHE = None


def _get_nc():
    global _NC_CACHE
    if _NC_CACHE is None:
        _NC_CACHE = build_bass()
    return _NC_CACHE


def _make_in_maps(preds, target, node_weights, feature_weights):
    in_maps = []
    for core in range(NCORES):
        lo, hi = core * LSH, (core + 1) * LSH
        p_sh = np.zeros((B, M, LPAD, V), dtype=np.float32)
        p_sh[:, :, :LSH] = preds[:, :, lo:hi]
        t_sh = np.zeros((B, LPAD, V), dtype=np.float32)
        t_sh[:, :LSH] = target[:, lo:hi]
        nw_sh = np.zeros((LPAD,), dtype=np.float32)
        nw_sh[:LSH] = node_weights[lo:hi]
        in_maps.append(
            {"preds": p_sh, "target": t_sh,
             "fw": np.ascontiguousarray(feature_weights, dtype=np.float32),
             "nw": nw_sh}
        )
    return in_maps


def kernel(preds, target, node_weights, feature_weights):
    preds = np.asarray(preds, dtype=np.float32)
    target = np.asarray(target, dtype=np.float32)
    node_weights = np.asarray(node_weights, dtype=np.float32)
    feature_weights = np.asarray(feature_weights, dtype=np.float32)

    nc = _get_nc()
    in_maps = _make_in_maps(preds, target, node_weights, feature_weights)

    res = run_bass_kernel_spmd(nc, in_maps, core_ids=list(range(NCORES)))
    total = np.float64(0.0)
    for r in res.results:
        total += np.asarray(r["out"], dtype=np.float64).sum()

    denom = 56.0 * np.float64(node_weights.astype(np.float64).sum()) * B
    return np.float32(total / denom)

